# revision 14
# baseline (speedup 1.0000x reference)
"""Trainium2 Bass kernel for causal multi-head attention (dense transformer block).

Problem (hardcoded): x [2, 2048, 1024], 16 heads x 64 dh, causal attention,
fp32 I/O. Sharding: 8 cores = 2 batches x 4 head-groups. Each core computes 4
heads for one batch plus a partial output projection [2048, 1024]; the host
sums the 4 partials per batch and adds b_O.

Everything on-device is computed in "transposed" orientation so no transposes
are needed anywhere:
  x^T (host-pretransposed)  ->  Q^T, K^T [dh, s] and V [s, dh] via matmuls
  S^T[k, q] = K Q^T         ->  P^T = exp(S^T / 8) (causal-masked pre-exp)
  Z^T[dh, q] = V^T P^T      ->  normalized by column sums (ones-matmul)
  O[s, :]   = (Z^T)^T W_O   (Z^T is directly the lhsT of the O-projection)

Heads are processed in pairs: QK^T packs 2 heads in row-groups (0-63 / 64-127)
of the PE array, PV packs 2 heads in column-groups -- both run concurrently.

Precision modes (ATTN_MM_DTYPE):
  fp32  -- everything fp32 (slowest, exact)
  mixed -- fp32r (11-bit mantissa) for projections/scores, bf16 for P*V
           (P-quantization cancels through the softmax normalization)
  bf16  -- everything bf16 (fastest)
"""

import os
from contextlib import ExitStack

import numpy as np

import concourse.tile as tile
from concourse import bacc, mybir
from concourse.bass_utils import run_bass_kernel_spmd

# problem constants
B, S, DM, H, DH = 2, 2048, 1024, 16, 64
P = 128          # partitions
QB = 512         # q block (matmul moving free dim)
NKT = S // P     # 16 k tiles
NQB = S // QB    # 4 q blocks
NDM = DM // P    # 8 d_model tiles
HPC = 4          # heads per core
NCORES = 8

F32 = mybir.dt.float32
BF16 = mybir.dt.bfloat16
F32R = mybir.dt.float32r

MM_DTYPE = os.environ.get("ATTN_MM_DTYPE", "bf16")

_PROGRAM_CACHE = {}
LAST_RESULTS = None  # BassKernelResults of the most recent run (for test.py)


def _mm(nc, out, lhsT, rhs, start, stop, skip=False):
    # skip_group_check: the sim's psum-group tracker doesn't distinguish
    # partition ranges; our concurrent groups in one bank are partition-disjoint
    # (rows 0-63 vs 64-127), which the per-partition zeroing model handles.
    nc.tensor.matmul(out, lhsT, rhs, start=start, stop=stop, skip_group_check=skip)


def build_program(mm_dtype=MM_DTYPE):
    """Build the single-core SPMD Bass program (same program on all 8 cores)."""
    if mm_dtype in _PROGRAM_CACHE:
        return _PROGRAM_CACHE[mm_dtype]

    # HI: projections + scores; LO: P and V (PV matmul)
    HI = {"fp32": F32, "mixed": F32R, "bf16": BF16}[mm_dtype]
    LO = {"fp32": F32, "mixed": BF16, "bf16": BF16}[mm_dtype]

    nc = bacc.Bacc(
        "TRN2", target_bir_lowering=False, debug=False, num_devices=NCORES
    )

    # ---- DRAM I/O (per-core shards, prearranged on host) ----
    xT_d = nc.dram_tensor("xT", [DM, S], HI, kind="ExternalInput")
    wq_d = nc.dram_tensor("wq", [DM, HPC * DH], HI, kind="ExternalInput")
    wk_d = nc.dram_tensor("wk", [DM, HPC * DH], HI, kind="ExternalInput")
    wv_d = nc.dram_tensor("wv", [DM, HPC * DH], HI, kind="ExternalInput")
    wo_d = nc.dram_tensor("wo", [HPC * DH, DM], HI, kind="ExternalInput")
    bq_d = nc.dram_tensor("bq", [2, P], F32, kind="ExternalInput")
    bk_d = nc.dram_tensor("bk", [2, P], F32, kind="ExternalInput")
    bv_d = nc.dram_tensor("bv", [P, HPC * DH], F32, kind="ExternalInput")
    bandm_d = nc.dram_tensor("bandm", [2, P, 2 * QB], LO, kind="ExternalInput")
    out_d = nc.dram_tensor("out", [S, DM], F32, kind="ExternalOutput")

    with tile.TileContext(nc) as tc, ExitStack() as ctx:
        const = ctx.enter_context(tc.tile_pool(name="const", bufs=1))
        persist = ctx.enter_context(tc.tile_pool(name="persist", bufs=1))

        # ---- constants ----
        ones64 = const.tile([P, 64], LO, name="ones64", tag="ones64")
        nc.gpsimd.memset(ones64[:], 1.0)
        bandm_sb = const.tile([P, 2, 2 * QB], LO, name="bandm_sb", tag="bandm")
        for o in range(2):
            nc.sync.dma_start(out=bandm_sb[:, o, :], in_=bandm_d[o, :, :])
        bq_sb = const.tile([P, 2], F32, name="bq_sb", tag="bq")
        bk_sb = const.tile([P, 2], F32, name="bk_sb", tag="bk")
        for p in range(2):
            nc.sync.dma_start(out=bq_sb[:, p : p + 1], in_=bq_d[p : p + 1, :])
            nc.sync.dma_start(out=bk_sb[:, p : p + 1], in_=bk_d[p : p + 1, :])
        bv_sb = const.tile([P, HPC * DH], F32, name="bv_sb", tag="bv")
        nc.sync.dma_start(out=bv_sb[:], in_=bv_d[:, :])

        # ---- persistent activations ----
        qt_sb = [
            persist.tile([P, S], HI, name=f"qt{p}", tag=f"qt{p}") for p in range(2)
        ]
        kt_sb = [
            persist.tile([P, S], HI, name=f"kt{p}", tag=f"kt{p}") for p in range(2)
        ]
        v_sb = [
            persist.tile([P, NKT, P], LO, name=f"v{p}", tag=f"v{p}")
            for p in range(2)
        ]
        zt_sb = [
            persist.tile([P, S], HI, name=f"zt{p}", tag=f"zt{p}") for p in range(2)
        ]
        wo_sb = persist.tile([P, 2, DM], HI, name="wo_sb", tag="wo")
        for p in range(2):
            nc.sync.dma_start(out=wo_sb[:, p, :], in_=wo_d[p * P : (p + 1) * P, :])

        # ======= single shared psum pool: projections, attention, O =======
        sp = ctx.enter_context(tc.tile_pool(name="sp", bufs=3, space="PSUM"))
        zp = ctx.enter_context(tc.tile_pool(name="zp", bufs=1, space="PSUM"))
        dp = ctx.enter_context(tc.tile_pool(name="dp", bufs=1, space="PSUM"))
        xw = ctx.enter_context(tc.tile_pool(name="xw", bufs=1))
        ppool = ctx.enter_context(
            tc.tile_pool(name="ppool", bufs=8 if LO == BF16 else 3)
        )
        bcpool = ctx.enter_context(tc.tile_pool(name="bcpool", bufs=2))
        ost = ctx.enter_context(tc.tile_pool(name="ost", bufs=4))

        # ---- input DMAs, interleaved so early k-tiles land first ----
        xt_sb = xw.tile([P, NDM, S], HI, name="xt_sb", tag="xt")
        w_sb = {
            wname: xw.tile([P, NDM, HPC * DH], HI, name=f"{wname}_sb", tag=wname)
            for wname in ("wq", "wk", "wv")
        }
        for t in range(NDM):
            nc.sync.dma_start(out=w_sb["wq"][:, t, :], in_=wq_d[t * P : (t + 1) * P, :])
            nc.sync.dma_start(out=xt_sb[:, t, :], in_=xT_d[t * P : (t + 1) * P, :])
        for t in range(NDM):
            nc.sync.dma_start(out=w_sb["wk"][:, t, :], in_=wk_d[t * P : (t + 1) * P, :])
            nc.sync.dma_start(out=w_sb["wv"][:, t, :], in_=wv_d[t * P : (t + 1) * P, :])

        def qk_proj(p):
            # Q^T and K^T for pair p: [dh-pair (128), seq]; rows 0-63 =
            # head 2p, 64-127 = head 2p+1
            for dst, wname, bias in (
                (qt_sb, "wq", bq_sb),
                (kt_sb, "wk", bk_sb),
            ):
                for ch in range(NQB):
                    qp = sp.tile([P, 2, QB], F32, name="qp", tag="s")
                    for t in range(NDM):
                        _mm(
                            nc,
                            qp[:, 0, :],
                            w_sb[wname][:, t, p * P : (p + 1) * P],
                            xt_sb[:, t, ch * QB : (ch + 1) * QB],
                            start=(t == 0),
                            stop=(t == NDM - 1),
                        )
                    nc.vector.tensor_scalar_add(
                        dst[p][:, ch * QB : (ch + 1) * QB],
                        qp[:, 0, :],
                        bias[:, p : p + 1],
                    )

        def v_proj():
            # V: [seq, head-pair dh] per 128-row tile, both pairs
            for st in range(NKT):
                vp = sp.tile([P, 2, QB], F32, name="vp", tag="s")
                for t in range(NDM):
                    _mm(
                        nc,
                        vp[:, 0, 0 : HPC * DH],
                        xt_sb[:, t, st * P : (st + 1) * P],
                        w_sb["wv"][:, t, :],
                        start=(t == 0),
                        stop=(t == NDM - 1),
                    )
                for p in range(2):
                    nc.vector.tensor_add(
                        v_sb[p][:, st, :],
                        vp[:, 0, p * P : (p + 1) * P],
                        bv_sb[:, p * P : (p + 1) * P],
                    )

        def attention(p):
            for qb in range(NQB):
                q0 = qb * QB
                nk = (qb + 1) * (QB // P)  # k tiles in causal range
                zps = zp.tile([P, QB], F32, name="zps", tag="z")
                dnb = dp.tile([P, QB], F32, name="dnb", tag="d")

                def pv_dnb(pA, pB, kg):
                    # PV (column-packed heads) + softmax denominators: the
                    # ones-matmul sums P over k AND broadcasts over the 64
                    # rows of each head half, accumulated in PSUM; all read
                    # only the valid q range of their k-tile
                    for j in range(2):
                        kt = kg * 2 + j
                        c0 = max(kt * P - q0, 0)
                        with tc.tile_critical():
                            _mm(
                                nc, zps[0:64, c0:QB], v_sb[p][:, kt, 0:64],
                                pA[:, j, c0:QB],
                                start=(kt == 0), stop=(kt == nk - 1), skip=True,
                            )
                            _mm(
                                nc, zps[64:P, c0:QB], v_sb[p][:, kt, 64:P],
                                pB[:, j, c0:QB],
                                start=(kt == 0), stop=(kt == nk - 1), skip=True,
                            )
                        with tc.tile_critical():
                            _mm(
                                nc, dnb[0:64, c0:QB], ones64[:], pA[:, j, c0:QB],
                                start=(kt == 0), stop=(kt == nk - 1), skip=True,
                            )
                            _mm(
                                nc, dnb[64:P, c0:QB], ones64[:], pB[:, j, c0:QB],
                                start=(kt == 0), stop=(kt == nk - 1), skip=True,
                            )

                for kg in range(nk // 2):
                    # offs[j]: first valid q column of k-tile kg*2+j
                    offs = [kg * 2 * P + j * P - q0 for j in range(2)]
                    band = offs[0] >= 0
                    deep = band and offs[0] >= 2 * P  # o=1 band k-group
                    sA = sp.tile([P, 2, QB], F32, name="sA", tag="s")
                    sB = sp.tile([P, 2, QB], F32, name="sB", tag="s")
                    for j in range(2):
                        # the deep band k-group only computes scores for the
                        # valid (unmasked) q columns
                        c0 = offs[j] if deep else 0
                        with tc.tile_critical():
                            for rows, stile in ((slice(0, 64), sA), (slice(64, P), sB)):
                                # 2 heads row-packed: A in partitions 0-63, B
                                # in 64-127, concurrent on the PE array
                                _mm(
                                    nc,
                                    stile[:, j, c0:QB],
                                    kt_sb[p][rows, (kg * 2 + j) * P : (kg * 2 + j + 1) * P],
                                    qt_sb[p][rows, q0 + c0 : q0 + QB],
                                    start=True,
                                    stop=True,
                                )
                    pA = ppool.tile([P, 2, QB], LO, name="pA", tag="pt")
                    pB = ppool.tile([P, 2, QB], LO, name="pB", tag="pt")
                    # exp(S/sqrt(dh)); scale folded into ACT
                    if deep:
                        # skip the fully-masked left part; zero it instead
                        for px, sx in ((pA, sA), (pB, sB)):
                            for j in range(2):
                                nc.vector.memset(px[:, j, 0 : offs[j]], 0.0)
                                nc.scalar.activation(
                                    px[:, j, offs[j] : QB],
                                    sx[:, j, offs[j] : QB],
                                    mybir.ActivationFunctionType.Exp,
                                    scale=0.125,
                                )
                    else:
                        nc.scalar.activation(
                            pA[:], sA[:], mybir.ActivationFunctionType.Exp,
                            scale=0.125,
                        )
                        nc.scalar.activation(
                            pB[:], sB[:], mybir.ActivationFunctionType.Exp,
                            scale=0.125,
                        )
                    if band:
                        # causal mask: multiply diagonal-band P tiles by 0/1
                        o = offs[0] // (2 * P)
                        nc.vector.tensor_mul(pA[:], pA[:], bandm_sb[:, o, :])
                        nc.vector.tensor_mul(pB[:], pB[:], bandm_sb[:, o, :])
                    pv_dnb(pA, pB, kg)

                bcs = bcpool.tile([P, QB], F32, name="bcs", tag="bcs")
                bcr = bcpool.tile([P, QB], F32, name="bcr", tag="bcr")
                nc.vector.reciprocal_approx_accurate(
                    out=bcr[:], in_=dnb[:], scratch=bcs[:]
                )
                nc.vector.tensor_mul(zt_sb[p][:, q0 : q0 + QB], zps[:], bcr[:])

        qk_proj(0)
        v_proj()
        qk_proj(1)
        attention(0)
        attention(1)

        # output projection tail; psum tiles round-robin across all pools
        # (5 slots) so the matmul/copy/DMA chain pipelines deeper
        pools = [sp, sp, sp, zp, dp]
        k = 0
        for st in range(NKT):
            for nn in range(2):
                pool = pools[k % len(pools)]
                k += 1
                shape = [P, 2, QB] if pool is sp else [P, QB]
                ops = pool.tile(shape, F32, name="ops",
                                tag="s" if pool is sp else ("z" if pool is zp else "d"))
                opsv = ops[:, 0, :] if pool is sp else ops[:]
                for pp in range(2):
                    _mm(
                        nc,
                        opsv,
                        zt_sb[pp][:, st * P : (st + 1) * P],
                        wo_sb[:, pp, nn * QB : (nn + 1) * QB],
                        start=(pp == 0),
                        stop=(pp == 1),
                    )
                ot = ost.tile([P, QB], F32, name="ot", tag="ot")
                if nn == 0:
                    nc.scalar.copy(ot[:], opsv)
                else:
                    nc.vector.tensor_copy(ot[:], opsv)
                nc.sync.dma_start(
                    out=out_d[st * P : (st + 1) * P, nn * QB : (nn + 1) * QB],
                    in_=ot[:],
                )

    nc.compile()
    _PROGRAM_CACHE[mm_dtype] = nc
    return nc


def _round_fp32r(a):
    """Round fp32 array to nearest fp32r (drop 12 low mantissa bits)."""
    u = np.ascontiguousarray(a, dtype=np.float32).view(np.uint32)
    u = ((u + 0x800) & np.uint32(0xFFFFF000)).astype(np.uint32)
    return u.view(np.float32)


def make_in_maps(
    normalized_resid_pre, W_Q, W_K, W_V, W_O, b_Q, b_K, b_V, b_O,
    mm_dtype=MM_DTYPE,
):
    """Shard + prearrange the full inputs into per-core input maps."""
    import ml_dtypes  # noqa: F401  (registers bfloat16 with numpy)

    np_hi = np.dtype("bfloat16") if mm_dtype == "bf16" else np.float32
    np_lo = np.float32 if mm_dtype == "fp32" else np.dtype("bfloat16")
    rnd = _round_fp32r if mm_dtype == "mixed" else (lambda a: a)

    x = np.asarray(normalized_resid_pre, dtype=np.float32)
    W_Q = np.asarray(W_Q, dtype=np.float32)
    W_K = np.asarray(W_K, dtype=np.float32)
    W_V = np.asarray(W_V, dtype=np.float32)
    W_O = np.asarray(W_O, dtype=np.float32)
    b_Q = np.asarray(b_Q, dtype=np.float32)
    b_K = np.asarray(b_K, dtype=np.float32)
    b_V = np.asarray(b_V, dtype=np.float32)

    xT = [rnd(np.ascontiguousarray(x[b].T)).astype(np_hi) for b in range(B)]
    # additive causal band masks at k-group granularity: variant o covers the
    # two k-tiles at q-block offsets (2o*128, (2o+1)*128)
    kp = np.arange(P)[:, None]
    qc = np.arange(QB)[None, :]
    bandm = np.stack(
        [
            np.concatenate(
                [
                    np.where(qc < (2 * o + j) * P + kp,
                             np.float32(0.0), np.float32(1.0))
                    for j in range(2)
                ],
                axis=1,
            )
            for o in range(2)
        ]
    ).astype(np_lo)

    in_maps = []
    for c in range(NCORES):
        b = c // (NCORES // B)
        heads = [HPC * (c % (NCORES // B)) + i for i in range(HPC)]
        wq = rnd(np.concatenate([W_Q[h] for h in heads], axis=1)).astype(np_hi)
        wk = rnd(np.concatenate([W_K[h] for h in heads], axis=1)).astype(np_hi)
        wv = rnd(np.concatenate([W_V[h] for h in heads], axis=1)).astype(np_hi)
        wo = rnd(np.concatenate([W_O[h] for h in heads], axis=0)).astype(np_hi)
        bq = np.stack(
            [
                np.concatenate([b_Q[heads[0]], b_Q[heads[1]]]),
                np.concatenate([b_Q[heads[2]], b_Q[heads[3]]]),
            ]
        ).astype(np.float32)
        bk = np.stack(
            [
                np.concatenate([b_K[heads[0]], b_K[heads[1]]]),
                np.concatenate([b_K[heads[2]], b_K[heads[3]]]),
            ]
        ).astype(np.float32)
        bv = np.tile(
            np.concatenate([b_V[h] for h in heads])[None, :], (P, 1)
        ).astype(np.float32)
        in_maps.append(
            {
                "xT": np.ascontiguousarray(xT[b]),
                "wq": wq, "wk": wk, "wv": wv, "wo": wo,
                "bq": bq, "bk": bk, "bv": bv,
                "bandm": bandm,
            }
        )
    return in_maps


def kernel(normalized_resid_pre, W_Q, W_K, W_V, W_O, b_Q, b_K, b_V, b_O):
    global LAST_RESULTS
    nc = build_program()
    in_maps = make_in_maps(
        normalized_resid_pre, W_Q, W_K, W_V, W_O, b_Q, b_K, b_V, b_O
    )
    trace = os.environ.get("ATTN_TRACE", "0") == "1"
    res = run_bass_kernel_spmd(nc, in_maps, list(range(NCORES)), trace=trace)
    LAST_RESULTS = res

    b_O = np.asarray(b_O, dtype=np.float32)
    parts = [np.asarray(res.results[c]["out"], dtype=np.float64) for c in range(NCORES)]
    npc = NCORES // B  # cores per batch
    out = np.stack(
        [sum(parts[b * npc : (b + 1) * npc]) + b_O for b in range(B)]
    )
    return out.astype(np.float32)


# revision 16
# speedup vs baseline: 3.2996x; 3.2996x over previous
"""Trainium2 Bass kernel for causal multi-head attention (dense transformer block).

Problem (hardcoded): x [2, 2048, 1024], 16 heads x 64 dh, causal attention,
fp32 I/O. Sharding: 8 cores = 2 batches x 4 head-groups. Each core computes 4
heads for one batch plus a partial output projection [2048, 1024]; the host
sums the 4 partials per batch and adds b_O.

Everything on-device is computed in "transposed" orientation so no transposes
are needed anywhere:
  x^T (host-pretransposed)  ->  Q^T, K^T [dh, s] and V [s, dh] via matmuls
  S^T[k, q] = K Q^T         ->  P^T = exp(S^T / 8) (causal-masked pre-exp)
  Z^T[dh, q] = V^T P^T      ->  normalized by column sums (ones-matmul)
  O[s, :]   = (Z^T)^T W_O   (Z^T is directly the lhsT of the O-projection)

Heads are processed in pairs: QK^T packs 2 heads in row-groups (0-63 / 64-127)
of the PE array, PV packs 2 heads in column-groups -- both run concurrently.

Precision modes (ATTN_MM_DTYPE):
  fp32  -- everything fp32 (slowest, exact)
  mixed -- fp32r (11-bit mantissa) for projections/scores, bf16 for P*V
           (P-quantization cancels through the softmax normalization)
  bf16  -- everything bf16 (fastest)
"""

import os
from contextlib import ExitStack

import numpy as np

import concourse.tile as tile
from concourse import bacc, mybir
from concourse.bass_utils import run_bass_kernel_spmd

# problem constants
B, S, DM, H, DH = 2, 2048, 1024, 16, 64
P = 128          # partitions
QB = 512         # q block (matmul moving free dim)
NKT = S // P     # 16 k tiles
NQB = S // QB    # 4 q blocks
NDM = DM // P    # 8 d_model tiles
HPC = 4          # heads per core
NCORES = 8

F32 = mybir.dt.float32
BF16 = mybir.dt.bfloat16
F32R = mybir.dt.float32r

MM_DTYPE = os.environ.get("ATTN_MM_DTYPE", "bf16")

_PROGRAM_CACHE = {}
LAST_RESULTS = None  # BassKernelResults of the most recent run (for test.py)


def _mm(nc, out, lhsT, rhs, start, stop, skip=False):
    # skip_group_check: the sim's psum-group tracker doesn't distinguish
    # partition ranges; our concurrent groups in one bank are partition-disjoint
    # (rows 0-63 vs 64-127), which the per-partition zeroing model handles.
    return nc.tensor.matmul(
        out, lhsT, rhs, start=start, stop=stop, skip_group_check=skip
    )


def _chain(insts):
    """Ordering-only PE edges so matmuls alternating between row/column
    groups stay adjacent and run concurrently on the array."""
    from concourse.tile import add_dep_helper

    for a, b in zip(insts[1:], insts):
        add_dep_helper(a.ins, b.ins, sync=False, reason="pack-pair order")


def build_program(mm_dtype=MM_DTYPE):
    """Build the single-core SPMD Bass program (same program on all 8 cores)."""
    if mm_dtype in _PROGRAM_CACHE:
        return _PROGRAM_CACHE[mm_dtype]

    # HI: projections + scores; LO: P and V (PV matmul)
    HI = {"fp32": F32, "mixed": F32R, "bf16": BF16}[mm_dtype]
    LO = {"fp32": F32, "mixed": BF16, "bf16": BF16}[mm_dtype]

    nc = bacc.Bacc(
        "TRN2", target_bir_lowering=False, debug=False, num_devices=NCORES
    )

    # ---- DRAM I/O (per-core shards, prearranged on host) ----
    xT_d = nc.dram_tensor("xT", [DM, S], HI, kind="ExternalInput")
    wq_d = nc.dram_tensor("wq", [DM, HPC * DH], HI, kind="ExternalInput")
    wk_d = nc.dram_tensor("wk", [DM, HPC * DH], HI, kind="ExternalInput")
    wv_d = nc.dram_tensor("wv", [DM, HPC * DH], HI, kind="ExternalInput")
    wo_d = nc.dram_tensor("wo", [HPC * DH, DM], HI, kind="ExternalInput")
    bq_d = nc.dram_tensor("bq", [2, P], F32, kind="ExternalInput")
    bk_d = nc.dram_tensor("bk", [2, P], F32, kind="ExternalInput")
    bv_d = nc.dram_tensor("bv", [P, HPC * DH], F32, kind="ExternalInput")
    bandm_d = nc.dram_tensor("bandm", [2, P, 2 * QB], LO, kind="ExternalInput")
    out_d = nc.dram_tensor("out", [S, DM], F32, kind="ExternalOutput")

    with tile.TileContext(nc) as tc, ExitStack() as ctx:
        const = ctx.enter_context(tc.tile_pool(name="const", bufs=1))
        persist = ctx.enter_context(tc.tile_pool(name="persist", bufs=1))

        # ---- constants ----
        ones64 = const.tile([P, 64], LO, name="ones64", tag="ones64")
        nc.gpsimd.memset(ones64[:], 1.0)
        bandm_sb = const.tile([P, 2, 2 * QB], LO, name="bandm_sb", tag="bandm")
        for o in range(2):
            nc.sync.dma_start(out=bandm_sb[:, o, :], in_=bandm_d[o, :, :])
        bq_sb = const.tile([P, 2], F32, name="bq_sb", tag="bq")
        bk_sb = const.tile([P, 2], F32, name="bk_sb", tag="bk")
        for p in range(2):
            nc.sync.dma_start(out=bq_sb[:, p : p + 1], in_=bq_d[p : p + 1, :])
            nc.sync.dma_start(out=bk_sb[:, p : p + 1], in_=bk_d[p : p + 1, :])
        bv_sb = const.tile([P, HPC * DH], F32, name="bv_sb", tag="bv")
        nc.sync.dma_start(out=bv_sb[:], in_=bv_d[:, :])

        # ---- persistent activations ----
        qt_sb = [
            persist.tile([P, S], HI, name=f"qt{p}", tag=f"qt{p}") for p in range(2)
        ]
        kt_sb = [
            persist.tile([P, S], HI, name=f"kt{p}", tag=f"kt{p}") for p in range(2)
        ]
        v_sb = [
            persist.tile([P, NKT, P], LO, name=f"v{p}", tag=f"v{p}")
            for p in range(2)
        ]
        zt_sb = [
            persist.tile([P, S], HI, name=f"zt{p}", tag=f"zt{p}") for p in range(2)
        ]
        wo_sb = persist.tile([P, 2, DM], HI, name="wo_sb", tag="wo")
        for p in range(2):
            nc.sync.dma_start(out=wo_sb[:, p, :], in_=wo_d[p * P : (p + 1) * P, :])

        # ======= single shared psum pool: projections, attention, O =======
        sp = ctx.enter_context(tc.tile_pool(name="sp", bufs=3, space="PSUM"))
        zp = ctx.enter_context(tc.tile_pool(name="zp", bufs=1, space="PSUM"))
        dp = ctx.enter_context(tc.tile_pool(name="dp", bufs=1, space="PSUM"))
        xw = ctx.enter_context(tc.tile_pool(name="xw", bufs=1))
        ppool = ctx.enter_context(
            tc.tile_pool(name="ppool", bufs=8 if LO == BF16 else 3)
        )
        bcpool = ctx.enter_context(tc.tile_pool(name="bcpool", bufs=2))
        ost = ctx.enter_context(tc.tile_pool(name="ost", bufs=4))

        # ---- input DMAs, interleaved so early k-tiles land first ----
        xt_sb = xw.tile([P, NDM, S], HI, name="xt_sb", tag="xt")
        w_sb = {
            wname: xw.tile([P, NDM, HPC * DH], HI, name=f"{wname}_sb", tag=wname)
            for wname in ("wq", "wk", "wv")
        }
        for t in range(NDM):
            nc.sync.dma_start(out=w_sb["wq"][:, t, :], in_=wq_d[t * P : (t + 1) * P, :])
            nc.sync.dma_start(out=xt_sb[:, t, :], in_=xT_d[t * P : (t + 1) * P, :])
        for t in range(NDM):
            nc.sync.dma_start(out=w_sb["wk"][:, t, :], in_=wk_d[t * P : (t + 1) * P, :])
            nc.sync.dma_start(out=w_sb["wv"][:, t, :], in_=wv_d[t * P : (t + 1) * P, :])

        def qk_proj(p):
            # Q^T and K^T for pair p: [dh-pair (128), seq]; rows 0-63 =
            # head 2p, 64-127 = head 2p+1
            for dst, wname, bias in (
                (qt_sb, "wq", bq_sb),
                (kt_sb, "wk", bk_sb),
            ):
                for ch in range(NQB):
                    qp = sp.tile([P, 2, QB], F32, name="qp", tag="s")
                    for t in range(NDM):
                        _mm(
                            nc,
                            qp[:, 0, :],
                            w_sb[wname][:, t, p * P : (p + 1) * P],
                            xt_sb[:, t, ch * QB : (ch + 1) * QB],
                            start=(t == 0),
                            stop=(t == NDM - 1),
                        )
                    nc.vector.tensor_scalar_add(
                        dst[p][:, ch * QB : (ch + 1) * QB],
                        qp[:, 0, :],
                        bias[:, p : p + 1],
                    )

        def v_proj():
            # V: [seq, head-pair dh] per 128-row tile, both pairs
            for st in range(NKT):
                vp = sp.tile([P, 2, QB], F32, name="vp", tag="s")
                for t in range(NDM):
                    _mm(
                        nc,
                        vp[:, 0, 0 : HPC * DH],
                        xt_sb[:, t, st * P : (st + 1) * P],
                        w_sb["wv"][:, t, :],
                        start=(t == 0),
                        stop=(t == NDM - 1),
                    )
                for p in range(2):
                    nc.vector.tensor_add(
                        v_sb[p][:, st, :],
                        vp[:, 0, p * P : (p + 1) * P],
                        bv_sb[:, p * P : (p + 1) * P],
                    )

        def attention(p):
            for qb in range(NQB):
                q0 = qb * QB
                nk = (qb + 1) * (QB // P)  # k tiles in causal range
                zps = zp.tile([P, QB], F32, name="zps", tag="z")
                dnb = dp.tile([P, QB], F32, name="dnb", tag="d")

                def pv_dnb(pA, pB, kg):
                    # PV (column-packed heads) + softmax denominators: the
                    # ones-matmul sums P over k AND broadcasts over the 64
                    # rows of each head half, accumulated in PSUM; all read
                    # only the valid q range of their k-tile
                    for j in range(2):
                        kt = kg * 2 + j
                        c0 = max(kt * P - q0, 0)
                        _chain([
                            _mm(
                                nc, zps[0:64, c0:QB], v_sb[p][:, kt, 0:64],
                                pA[:, j, c0:QB],
                                start=(kt == 0), stop=(kt == nk - 1), skip=True,
                            ),
                            _mm(
                                nc, zps[64:P, c0:QB], v_sb[p][:, kt, 64:P],
                                pB[:, j, c0:QB],
                                start=(kt == 0), stop=(kt == nk - 1), skip=True,
                            ),
                            _mm(
                                nc, dnb[0:64, c0:QB], ones64[:], pA[:, j, c0:QB],
                                start=(kt == 0), stop=(kt == nk - 1), skip=True,
                            ),
                            _mm(
                                nc, dnb[64:P, c0:QB], ones64[:], pB[:, j, c0:QB],
                                start=(kt == 0), stop=(kt == nk - 1), skip=True,
                            ),
                        ])

                for kg in range(nk // 2):
                    # offs[j]: first valid q column of k-tile kg*2+j
                    offs = [kg * 2 * P + j * P - q0 for j in range(2)]
                    band = offs[0] >= 0
                    deep = band and offs[0] >= 2 * P  # o=1 band k-group
                    sA = sp.tile([P, 2, QB], F32, name="sA", tag="s")
                    sB = sp.tile([P, 2, QB], F32, name="sB", tag="s")
                    for j in range(2):
                        # the deep band k-group only computes scores for the
                        # valid (unmasked) q columns
                        c0 = offs[j] if deep else 0
                        _chain([
                            _mm(
                                nc,
                                stile[:, j, c0:QB],
                                kt_sb[p][rows, (kg * 2 + j) * P : (kg * 2 + j + 1) * P],
                                qt_sb[p][rows, q0 + c0 : q0 + QB],
                                start=True,
                                stop=True,
                            )
                            for rows, stile in ((slice(0, 64), sA), (slice(64, P), sB))
                        ])
                    pA = ppool.tile([P, 2, QB], LO, name="pA", tag="pt")
                    pB = ppool.tile([P, 2, QB], LO, name="pB", tag="pt")
                    # exp(S/sqrt(dh)); scale folded into ACT
                    if deep:
                        # skip the fully-masked left part; zero it instead
                        for px, sx in ((pA, sA), (pB, sB)):
                            for j in range(2):
                                nc.vector.memset(px[:, j, 0 : offs[j]], 0.0)
                                nc.scalar.activation(
                                    px[:, j, offs[j] : QB],
                                    sx[:, j, offs[j] : QB],
                                    mybir.ActivationFunctionType.Exp,
                                    scale=0.125,
                                )
                    else:
                        nc.scalar.activation(
                            pA[:], sA[:], mybir.ActivationFunctionType.Exp,
                            scale=0.125,
                        )
                        nc.scalar.activation(
                            pB[:], sB[:], mybir.ActivationFunctionType.Exp,
                            scale=0.125,
                        )
                    if band:
                        # causal mask: multiply diagonal-band P tiles by 0/1
                        o = offs[0] // (2 * P)
                        nc.vector.tensor_mul(pA[:], pA[:], bandm_sb[:, o, :])
                        nc.vector.tensor_mul(pB[:], pB[:], bandm_sb[:, o, :])
                    pv_dnb(pA, pB, kg)

                bcs = bcpool.tile([P, QB], F32, name="bcs", tag="bcs")
                bcr = bcpool.tile([P, QB], F32, name="bcr", tag="bcr")
                nc.vector.reciprocal_approx_accurate(
                    out=bcr[:], in_=dnb[:], scratch=bcs[:]
                )
                nc.vector.tensor_mul(zt_sb[p][:, q0 : q0 + QB], zps[:], bcr[:])

        qk_proj(0)
        v_proj()
        qk_proj(1)
        attention(0)
        attention(1)

        # output projection tail; psum tiles round-robin across all pools
        # (5 slots) so the matmul/copy/DMA chain pipelines deeper
        pools = [sp, sp, sp, zp, dp]
        k = 0
        for st in range(NKT):
            for nn in range(2):
                pool = pools[k % len(pools)]
                k += 1
                shape = [P, 2, QB] if pool is sp else [P, QB]
                ops = pool.tile(shape, F32, name="ops",
                                tag="s" if pool is sp else ("z" if pool is zp else "d"))
                opsv = ops[:, 0, :] if pool is sp else ops[:]
                for pp in range(2):
                    _mm(
                        nc,
                        opsv,
                        zt_sb[pp][:, st * P : (st + 1) * P],
                        wo_sb[:, pp, nn * QB : (nn + 1) * QB],
                        start=(pp == 0),
                        stop=(pp == 1),
                    )
                ot = ost.tile([P, QB], F32, name="ot", tag="ot")
                if nn == 0:
                    nc.scalar.copy(ot[:], opsv)
                else:
                    nc.vector.tensor_copy(ot[:], opsv)
                nc.sync.dma_start(
                    out=out_d[st * P : (st + 1) * P, nn * QB : (nn + 1) * QB],
                    in_=ot[:],
                )

    nc.compile()
    _PROGRAM_CACHE[mm_dtype] = nc
    return nc


def _round_fp32r(a):
    """Round fp32 array to nearest fp32r (drop 12 low mantissa bits)."""
    u = np.ascontiguousarray(a, dtype=np.float32).view(np.uint32)
    u = ((u + 0x800) & np.uint32(0xFFFFF000)).astype(np.uint32)
    return u.view(np.float32)


def make_in_maps(
    normalized_resid_pre, W_Q, W_K, W_V, W_O, b_Q, b_K, b_V, b_O,
    mm_dtype=MM_DTYPE,
):
    """Shard + prearrange the full inputs into per-core input maps."""
    import ml_dtypes  # noqa: F401  (registers bfloat16 with numpy)

    np_hi = np.dtype("bfloat16") if mm_dtype == "bf16" else np.float32
    np_lo = np.float32 if mm_dtype == "fp32" else np.dtype("bfloat16")
    rnd = _round_fp32r if mm_dtype == "mixed" else (lambda a: a)

    x = np.asarray(normalized_resid_pre, dtype=np.float32)
    W_Q = np.asarray(W_Q, dtype=np.float32)
    W_K = np.asarray(W_K, dtype=np.float32)
    W_V = np.asarray(W_V, dtype=np.float32)
    W_O = np.asarray(W_O, dtype=np.float32)
    b_Q = np.asarray(b_Q, dtype=np.float32)
    b_K = np.asarray(b_K, dtype=np.float32)
    b_V = np.asarray(b_V, dtype=np.float32)

    xT = [rnd(np.ascontiguousarray(x[b].T)).astype(np_hi) for b in range(B)]
    # additive causal band masks at k-group granularity: variant o covers the
    # two k-tiles at q-block offsets (2o*128, (2o+1)*128)
    kp = np.arange(P)[:, None]
    qc = np.arange(QB)[None, :]
    bandm = np.stack(
        [
            np.concatenate(
                [
                    np.where(qc < (2 * o + j) * P + kp,
                             np.float32(0.0), np.float32(1.0))
                    for j in range(2)
                ],
                axis=1,
            )
            for o in range(2)
        ]
    ).astype(np_lo)

    in_maps = []
    for c in range(NCORES):
        b = c // (NCORES // B)
        heads = [HPC * (c % (NCORES // B)) + i for i in range(HPC)]
        wq = rnd(np.concatenate([W_Q[h] for h in heads], axis=1)).astype(np_hi)
        wk = rnd(np.concatenate([W_K[h] for h in heads], axis=1)).astype(np_hi)
        wv = rnd(np.concatenate([W_V[h] for h in heads], axis=1)).astype(np_hi)
        wo = rnd(np.concatenate([W_O[h] for h in heads], axis=0)).astype(np_hi)
        bq = np.stack(
            [
                np.concatenate([b_Q[heads[0]], b_Q[heads[1]]]),
                np.concatenate([b_Q[heads[2]], b_Q[heads[3]]]),
            ]
        ).astype(np.float32)
        bk = np.stack(
            [
                np.concatenate([b_K[heads[0]], b_K[heads[1]]]),
                np.concatenate([b_K[heads[2]], b_K[heads[3]]]),
            ]
        ).astype(np.float32)
        bv = np.tile(
            np.concatenate([b_V[h] for h in heads])[None, :], (P, 1)
        ).astype(np.float32)
        in_maps.append(
            {
                "xT": np.ascontiguousarray(xT[b]),
                "wq": wq, "wk": wk, "wv": wv, "wo": wo,
                "bq": bq, "bk": bk, "bv": bv,
                "bandm": bandm,
            }
        )
    return in_maps


def kernel(normalized_resid_pre, W_Q, W_K, W_V, W_O, b_Q, b_K, b_V, b_O):
    global LAST_RESULTS
    nc = build_program()
    in_maps = make_in_maps(
        normalized_resid_pre, W_Q, W_K, W_V, W_O, b_Q, b_K, b_V, b_O
    )
    trace = os.environ.get("ATTN_TRACE", "0") == "1"
    res = run_bass_kernel_spmd(nc, in_maps, list(range(NCORES)), trace=trace)
    LAST_RESULTS = res

    b_O = np.asarray(b_O, dtype=np.float32)
    parts = [np.asarray(res.results[c]["out"], dtype=np.float64) for c in range(NCORES)]
    npc = NCORES // B  # cores per batch
    out = np.stack(
        [sum(parts[b * npc : (b + 1) * npc]) + b_O for b in range(B)]
    )
    return out.astype(np.float32)
